# revision 23
# baseline (speedup 1.0000x reference)
"""GQA kernel for Trainium2 (Bass/Tile), 8-core head-parallel. v3.

Problem: x(1,2048,1024), Wq(1024,1024)+bq, Wk/Wv(1024,256)+bk/bv,
16 Q heads / 4 KV heads, head_dim 64, full (non-causal) softmax attention.
Reference output is attn(B,H,S,Dh) reshaped DIRECTLY to (B,S,H*Dh):
out rows [h*128,(h+1)*128) of the (2048,1024) output belong to head h.

Sharding: core d owns Q heads {2d, 2d+1} (both share KV head d//2), so each
core computes a contiguous (256,1024) slab of the final output.

Host-side prep (free): x transposed+cast to bf16 xT (1024,2048); per-core
weight slices pre-scaled (Wq/8 folds 1/sqrt(64)) and packed Wkv=[Wk|Wv],
all cast to bf16.

v3 structure (v2 measured 115016ns; trace analysis):
  - v2 timeline: first ACTIVATE at 23.2us, 15.8us of exp gaps, 7.9us tail.
    PE union busy 85.8us (nearly as loaded as scalar's 71us of exp).
  - HAM pstate ramp: PE runs at 1.2GHz until ~12us after its first matmul
    (ham k=8/8 at t=21.1us in v2). -> warmup now starts at ~6.7us with ZERO
    dependencies (garbage SBUF weights, no ident2/memset waits).
  - DMA rebalance: wkv on the fast gpsimd queue (~208GB/s) first, wq early
    on scalar; xT block 0 on sync(0:2,2:4 strips)+gpsimd(4:8). Weights were
    v2's critical path (wkv landed 12.6us on the slow scalar queue).
  - PV pair merged into ONE matmul: both Q heads share the SAME KV head,
    so PV h0/h1 share the stationary v'. pso is one [65,1024] psum tile
    (2 banks, same total as the old pair), halving PV instruction count.
  - q0's PV(3..15) are DEFERRED out of the wave era (the waves were PE-
    oversubscribed: proj+scores+PV > 4.6us of exp per wave). They run as
    "carry" work spread <=2 per kb at the start of the q1 era, where the
    PE has ~8us of slack. pt ring holds them (bufs=16, +3MB SBUF).
  - proj_q(1..3) moved INTO wave 1..3 as two 4-MM half-bursts between
    score emits (v2 ran them as 8-MM bursts inside the q1/q2 exp eras,
    costing 2x2us exp gaps).
  - vtr(1..3) (V transposes) also deferred into the q1 era (only PV needs
    them).
  - Era PV schedule: carry PVs (<=2/kb) -> prev-era output once carry
    drains -> own PVs catch up (<=2/kb, lag>=1). Keeps per-kb PE work
    under the 1.147us exp period so the scalar stream never starves.
  - Output in bf16 (host casts back to f32; ~0.2% extra rounding, gate is
    2e-2): halves the output DMA. Final era's h1 DMA goes on the idle
    gpsimd queue.
  - PSUM budget 8 banks: scores 2x[128,1024] (4) + pso [65,1024] (2) +
    proj/transpose/dup ring (2).
"""

import numpy as np

import concourse.bass as bass
import concourse.mybir as mybir
import concourse.tile as tile
from concourse import bacc
from concourse.bass_utils import run_bass_kernel_spmd
from concourse.masks import make_identity

F32 = mybir.dt.float32
BF16 = mybir.dt.bfloat16
F16 = mybir.dt.float16
I16 = mybir.dt.int16
AF = mybir.ActivationFunctionType
ALU = mybir.AluOpType

S = 2048
DIM = 1024
HD = 64
N_CORES = 8
NCH = DIM // 128   # 8 contraction chunks

SHIFT = -2.0                      # exp(s+SHIFT), cancels in softmax


def build_kernel():
    nc = bacc.Bacc("TRN2", target_bir_lowering=False, debug=False, num_devices=N_CORES)

    # weights host-prearranged to [128, chunk, 128] so the DMA is contiguous
    xt_d = nc.dram_tensor("xt", [DIM, S], BF16, kind="ExternalInput").ap()
    wq_d = nc.dram_tensor("wq", [128, NCH, 128], BF16, kind="ExternalInput").ap()
    wkv_d = nc.dram_tensor("wkv", [128, NCH, 128], BF16, kind="ExternalInput").ap()
    b_d = nc.dram_tensor("b", [128, 2], F32, kind="ExternalInput").ap()
    o_d = nc.dram_tensor("o", [2, S, HD], BF16, kind="ExternalOutput").ap()

    with tile.TileContext(nc) as tc:
        with (
            tc.tile_pool(name="const", bufs=1) as const_pool,
            tc.tile_pool(name="persist", bufs=1) as persist_pool,
            tc.tile_pool(name="pt", bufs=20) as pt_pool,
            tc.tile_pool(name="outs", bufs=2) as out_pool,
            tc.tile_pool(name="ps_s", bufs=2, space="PSUM") as ps_s,
            tc.tile_pool(name="ps_o", bufs=1, space="PSUM") as ps_o,
            tc.tile_pool(name="ps_m", bufs=2, space="PSUM") as ps_m,
        ):
            # ---- persistent SBUF ----
            xT = persist_pool.tile([128, NCH, S], BF16)    # 4 MB
            qt_sb = persist_pool.tile([128, S], BF16)      # rows h*64+d
            kv_sb = persist_pool.tile([128, S], BF16)      # 0:64 KT, 64:128 VT
            kt2u = persist_pool.tile([128, S], BF16)       # KT dup at rows 64:128
            v_sb = persist_pool.tile([128, 16, 65], BF16)  # V' chunks + ones col

            # ---- PE warmup: FIRST PE instructions, zero dependencies.
            # Garbage SBUF as weights+moving; results discarded. Purpose is
            # only to start the HAM pstate ramp (~12us to full clock) ASAP
            # and keep the PE busy until block-0 data lands (~9.2us).
            for w in range(2):
                warm = ps_m.tile([64, 256], F32, tag="proj")
                for r in range(18):
                    nc.tensor.matmul(warm[:], qt_sb[0:64, 0:64],
                                     qt_sb[0:64, 0:256],
                                     start=(r == 0), stop=(r == 17),
                                     skip_group_check=True)

            # ---- input DMAs ----
            # Measured queue rates (v3 trace, concurrent): gpsimd ~100 GB/s,
            # sync ~70, scalar ~68; the vector queue is a 4th stream.
            # dma_start only ISSUES (~0.7us on the sequencer); the DGE queue
            # streams in the background with ~2.5us start latency. Critical
            # path: wkv (split across the 2 fastest queues) -> kv quarter 1,
            # wq -> q chain. xT block 0 spread over all 4 queues.
            wq_sb = const_pool.tile([128, NCH, 128], BF16)
            wkv_sb = const_pool.tile([128, NCH, 128], BF16)
            b_sb = const_pool.tile([128, 2], F32)
            nc.scalar.dma_start(b_sb[:], b_d[:])
            nc.gpsimd.dma_start(wkv_sb[:, 0:4, :], wkv_d[:, 0:4, :])
            nc.sync.dma_start(wkv_sb[:, 4:8, :], wkv_d[:, 4:8, :])
            nc.scalar.dma_start(wq_sb[:], wq_d[:])
            bq_sb = b_sb[:, 0:1]
            bkv_sb = b_sb[:, 1:2]

            xt4 = xt_d.rearrange("(g p) s -> p g s", p=128)  # g: 8 chunks
            s0 = slice(0, 512)
            nc.sync.dma_start(xT[:, 0:2, s0], xt4[:, 0:2, s0])
            nc.gpsimd.dma_start(xT[:, 2:6, s0], xt4[:, 2:6, s0])
            nc.scalar.dma_start(xT[:, 6:8, s0], xt4[:, 6:8, s0])
            B0_ORDER = (0, 1, 2, 3, 4, 5, 6, 7)
            for bb in range(1, 4):
                sl = slice(bb * 512, (bb + 1) * 512)
                nc.scalar.dma_start(xT[:, 0:2, sl], xt4[:, 0:2, sl])
                nc.sync.dma_start(xT[:, 2:4, sl], xt4[:, 2:4, sl])
                nc.gpsimd.dma_start(xT[:, 4:8, sl], xt4[:, 4:8, sl])

            # small consts on vector (queues stay clear for weights/xT)
            for kb in range(16):
                nc.vector.memset(v_sb[:, kb, 64:65], 1.0)
            shift_sb = const_pool.tile([128, 1], F32)
            nc.vector.memset(shift_sb[:], SHIFT)

            # ---- identity (gpsimd emits it AFTER its DMA issues; needed
            # only from dup/vtr at ~15us). ident2 rows 64:128 come from the
            # diagonal block of ident via a same-partition DVE copy (v2 used
            # an SBUF->SBUF DMA on the now-busy sync queue).
            ident = const_pool.tile([128, 128], F32)
            make_identity(nc, ident[:])
            ident2 = const_pool.tile([128, 64], BF16)
            nc.vector.tensor_copy(ident2[0:64, :], ident[0:64, 0:64])
            nc.vector.tensor_copy(ident2[64:128, :], ident[64:128, 64:128])
            identb = const_pool.tile([128, 128], BF16)
            nc.vector.tensor_copy(identb[:], ident[:])

            # ---- helpers ----
            def proj_kv(bb, order=tuple(range(NCH)), cols=slice(0, 512)):
                # cols: column sub-range of the block (kb granularity), used
                # to get kb0's K out ~2.5us earlier during the slow-pstate
                # startup
                lo = bb * 512 + cols.start
                sl = slice(lo, bb * 512 + cols.stop)
                n = cols.stop - cols.start
                pskv = ps_m.tile([128, 512], F32, tag="proj")
                for i, c in enumerate(order):
                    nc.tensor.matmul(pskv[:, 0:n], wkv_sb[:, c, :],
                                     xT[:, c, sl],
                                     start=(i == 0), stop=(i == NCH - 1))
                nc.vector.tensor_scalar_add(kv_sb[:, sl], pskv[:, 0:n],
                                            bkv_sb[:])

            def emit_dup(bb, cols=slice(0, 512)):
                # kt2u dup: col-tiled PE matmul (I64 @ K -> partitions
                # 64:128) + DVE copy -- the DMA queues are saturated with xT.
                sl = slice(bb * 512 + cols.start, bb * 512 + cols.stop)
                n = cols.stop - cols.start
                psd = ps_m.tile([128, 512], F32, tag="proj")
                nc.tensor.matmul(psd[64:128, 0:n], ident2[0:64, :],
                                 kv_sb[0:64, sl], start=True, stop=True)
                nc.vector.tensor_copy(kt2u[64:128, sl], psd[64:128, 0:n])

            psq_pend = {}  # bb -> partially accumulated psq tile

            def proj_q(bb, order=tuple(range(NCH)), part=None):
                # part=0/1 emits one 4-MM half-burst (kept under the score
                # ring's ~1.2us absorption so the exp stream never stalls);
                # part=None emits the whole projection.
                sl = slice(bb * 512, (bb + 1) * 512)
                if part == 1:
                    psq = psq_pend.pop(bb)
                else:
                    psq = ps_m.tile([128, 512], F32, tag="proj")
                cs = order if part is None else order[part * 4:part * 4 + 4]
                for i, c in enumerate(cs):
                    first = (part != 1) and i == 0
                    last = (part != 0) and i == len(cs) - 1
                    nc.tensor.matmul(psq[:], wq_sb[:, c, :], xT[:, c, sl],
                                     start=first, stop=last,
                                     skip_group_check=True)
                if part == 0:
                    psq_pend[bb] = psq
                else:
                    nc.vector.tensor_scalar_add(qt_sb[:, sl], psq[:], bq_sb[:])

            def vtr1(kb):
                ps = ps_m.tile([128, 64], BF16, tag="proj")
                nc.tensor.matmul(
                    ps[:], kv_sb[64:128, kb * 128:(kb + 1) * 128],
                    ident2[64:128, :], is_transpose=True)
                nc.vector.tensor_copy(v_sb[:, kb, 0:64], ps[:])

            def emit_scores(qsl, kb):
                """score pair for (h0,h1) at k-block kb -> [128,1024] psum."""
                pss = ps_s.tile([128, 1024], F32, tag="s")
                kcols = slice(kb * 128, (kb + 1) * 128)
                nc.tensor.matmul(pss[:, 0:512], kv_sb[0:64, kcols],
                                 qt_sb[0:64, qsl], start=True, stop=True)
                nc.tensor.matmul(pss[:, 512:1024], kt2u[64:128, kcols],
                                 qt_sb[64:128, qsl], start=True, stop=True)
                return pss

            def emit_exp(pss, qb, kb):
                pt = pt_pool.tile([128, 1024], F16)
                nc.scalar.activation(pt[:], pss[:], AF.Exp, bias=shift_sb[:])
                return pt

            def emit_pv(pso, pt, kb):
                # both heads share V' (same KV head); ISA caps a matmul's
                # moving size at 512 cols, so two MMs into one psum tile
                nc.tensor.matmul(pso[:, 0:512], v_sb[:, kb, :], pt[:, 0:512],
                                 start=(kb == 0), stop=(kb == 15),
                                 skip_group_check=True)
                nc.tensor.matmul(pso[:, 512:1024], v_sb[:, kb, :],
                                 pt[:, 512:1024],
                                 start=(kb == 0), stop=(kb == 15),
                                 skip_group_check=True)

            def emit_output(qb, pso, final=False):
                # ot in bf16: halves the PE transpose cost (1 cyc/row vs 2
                # for f32); adds ~0.4% rounding on numerator+denominator.
                # The final era's output is pipelined in 256-col quarters
                # (DVE cast -> PE transpose -> DVE rcp/mult -> DMA) across
                # both idle queues to shrink the serial tail.
                qsl = slice(qb * 512, (qb + 1) * 512)
                nq = 4 if final else 1
                w = 512 // nq
                nt = w // 128  # transposes per piece
                for h in range(2):
                    for q in range(nq):
                        cl = slice(h * 512 + q * w, h * 512 + (q + 1) * w)
                        ot_sb = out_pool.tile([65, w], BF16,
                                              tag=f"ot{h}{q}n{nq}")
                        nc.vector.tensor_copy(ot_sb[:], pso[:, cl])
                        ps = ps_m.tile([128, nt, 66], BF16, tag="proj")
                        for j in range(nt):
                            nc.tensor.transpose(
                                ps[:, j, 0:65],
                                ot_sb[:, j * 128:(j + 1) * 128],
                                identb[:65, :65])
                        rcp = out_pool.tile([128, nt, 1], F32,
                                            tag=f"rcp{h}{q}n{nq}")
                        nc.vector.reciprocal(rcp[:], ps[:, :, 64:65])
                        o_sb = out_pool.tile([128, nt, HD], BF16,
                                             tag=f"o{h}{q}n{nq}")
                        nc.vector.tensor_tensor(
                            o_sb[:], ps[:, :, 0:64],
                            rcp[:].broadcast_to([128, nt, HD]),
                            mybir.AluOpType.mult)
                        eng = nc.gpsimd if (final and (h + q) % 2) else nc.sync
                        rsl = slice(qsl.start + q * w, qsl.start + (q + 1) * w)
                        eng.dma_start(
                            o_d[h, rsl, :].rearrange("(t j) c -> j t c",
                                                     j=128),
                            o_sb[:])

            # ---- unified deferred-work fifo ----
            # Items: ("pv", pso, pt, kb, ready_seq) and ("out", qb, pso).
            # ALL PVs are deferred into the fifo; the q1..q3 eras drain it
            # adaptively (<=2 PV-pairs per kb, more when backlogged) so the
            # exp stream paces the kernel and the PE never falls behind
            # locally. A PV is only popped once its exp is at least one kb
            # in the past (lag>=1), else the in-order PE FIFO would stall
            # on the activation.
            fifo = []
            nseq = [0]

            def drain(kb, qb):
                budget = 2 if (len(fifo) > 6
                               or (qb == 3 and len(fifo) > 15 - kb)) else 1
                popped = 0
                while fifo and popped < budget:
                    it = fifo[0]
                    if it[0] == "pv":
                        if it[4] > nseq[0] - 1:
                            break  # too fresh: exp still in flight
                        fifo.pop(0)
                        emit_pv(it[1], it[2], it[3])
                        popped += 1
                    else:
                        if popped:
                            break  # output starts a fresh kb slot
                        fifo.pop(0)
                        emit_output(it[1], it[2])
                        popped = 2

            # ---- B(0): kv in kb-quarters so kb0's K is ready before the
            # full q chain; q0 scores start earlier at MID pstate ----
            q0 = slice(0, 512)
            pso = ps_o.tile([65, 1024], F32, tag="o")
            proj_kv(0, B0_ORDER, cols=slice(0, 128))
            emit_dup(0, cols=slice(0, 128))
            vtr1(0)
            proj_q(0, B0_ORDER)
            for qq in range(1, 4):
                proj_kv(0, B0_ORDER, cols=slice(qq * 128, qq * 128 + 128))
                emit_dup(0, cols=slice(qq * 128, qq * 128 + 128))
                vtr1(qq)

            # ---- q0 wave pipeline: scores+exp only; kv(bb)/proj_q(bb)
            # spread through the waves; vtr 1/kb; PVs all into the fifo ----
            for bb in range(4):
                if bb > 0:
                    proj_kv(bb)
                    emit_dup(bb)
                for j, kb in enumerate(range(bb * 4, bb * 4 + 4)):
                    pss = emit_scores(q0, kb)
                    pt = emit_exp(pss, 0, kb)
                    fifo.append(("pv", pso, pt, kb, nseq[0] + 1))
                    nseq[0] += 1
                    if kb >= 4:
                        vtr1(kb)
                    if bb > 0 and j == 1:
                        proj_q(bb, part=0)
                    if bb > 0 and j == 2:
                        proj_q(bb, part=1)
            fifo.append(("out", 0, pso))

            # ---- exp-paced eras q1..q3 ----
            for qb in range(1, 4):
                qsl = slice(qb * 512, (qb + 1) * 512)
                pso = ps_o.tile([65, 1024], F32, tag="o")
                for kb in range(16):
                    pss = emit_scores(qsl, kb)
                    pt = emit_exp(pss, qb, kb)
                    fifo.append(("pv", pso, pt, kb, nseq[0] + 1))
                    nseq[0] += 1
                    drain(kb, qb)
                fifo.append(("out", qb, pso))
            while fifo:
                it = fifo.pop(0)
                if it[0] == "pv":
                    emit_pv(it[1], it[2], it[3])
                else:
                    emit_output(it[1], it[2], final=(not fifo))

    nc.compile()
    return nc


_NC_CACHE = None


def make_in_maps(inputs):
    import ml_dtypes
    x = np.asarray(inputs["x"], np.float32).reshape(S, DIM)
    xt = np.ascontiguousarray(x.T).astype(ml_dtypes.bfloat16)
    Wq = np.asarray(inputs["Wq"], np.float32)
    bq = np.asarray(inputs["bq"], np.float32)
    Wk = np.asarray(inputs["Wk"], np.float32)
    bk = np.asarray(inputs["bk"], np.float32)
    Wv = np.asarray(inputs["Wv"], np.float32)
    bv = np.asarray(inputs["bv"], np.float32)

    in_maps = []
    for d in range(N_CORES):
        g = d // 2
        wkv = np.concatenate(
            [Wk[:, g * 64:(g + 1) * 64], Wv[:, g * 64:(g + 1) * 64]], axis=1)
        bkv = np.concatenate([bk[g * 64:(g + 1) * 64], bv[g * 64:(g + 1) * 64]])
        wq_s = (Wq[:, d * 128:(d + 1) * 128] / 8.0).astype(ml_dtypes.bfloat16)
        wkv_s = wkv.astype(ml_dtypes.bfloat16)
        b2 = np.stack([bq[d * 128:(d + 1) * 128] / 8.0, bkv], axis=1)
        in_maps.append({
            "xt": xt,
            # [1024,128] -> [128 partition, 8 chunk, 128] contiguous
            "wq": np.ascontiguousarray(wq_s.reshape(NCH, 128, 128).transpose(1, 0, 2)),
            "wkv": np.ascontiguousarray(wkv_s.reshape(NCH, 128, 128).transpose(1, 0, 2)),
            "b": np.ascontiguousarray(b2, dtype=np.float32),
        })
    return in_maps


def kernel(**inputs) -> np.ndarray:
    global _NC_CACHE
    if _NC_CACHE is None:
        _NC_CACHE = build_kernel()
    nc = _NC_CACHE
    in_maps = make_in_maps(inputs)
    res = run_bass_kernel_spmd(nc, in_maps, list(range(N_CORES)))
    blocks = [np.asarray(res.results[d]["o"]).astype(np.float32).reshape(256, DIM)
              for d in range(N_CORES)]
    return np.concatenate(blocks, axis=0).reshape(1, S, DIM).astype(np.float32)
